# revision 33
# baseline (speedup 1.0000x reference)
"""Trainium2 Bass kernel for nn_ContentMultiheadAttention_523986010170.

Full (unsharded) inputs in, full output out. Internally shards across 8
NeuronCores: core c handles batch b = c//2 and query-row half c%2 (1024 of
2048 rows), computing all 8 heads for its slice. Outputs are disjoint
[1024, 512] blocks of the [S, B, E] result, gathered on the host.

v4 design (ACT-paced, fully decoupled): the scalar engine's 128 score
exps + 8 mask exps (~163us busy) are the hard floor; everything else is
scheduled to keep that stream dense and the PE clock warm:

  - 2-head groups x 16 t-blocks: one [128,1024] f32 score psum + one exp
    per iteration.  PSUM = 4 banks (scores x2) + 2 (AV accum) + 2 (proj).
  - rowsum folded into the AV matmul: lhsT = [v_h | ones] (M=65), so row
    64 of the AV accumulator is the softmax denominator.
  - A = exp(scores) * exp(mask): one [128,1024] multiply per iteration
    with the mask factor free-dim-broadcast; split DVE(12/16)/GPSIMD(4/16).
  - work that trails a group (lagged AV matmuls, the normalize epilogue)
    carries into later iterations via a global due-iteration queue, so no
    engine queue head-of-line blocks at group boundaries.
  - NO gpsimd partition_broadcast: it lives in a different Q7 library
    than tensor_tensor and every switch costs ~6-8us of IRAM reload (the
    dominant stall of the original kernel).  The recip row is broadcast
    with a K=1 PE matmul (ones-row (x) bf16-recip-row -> PSUM) instead.
  - out-proj reads head PAIRS stacked on 128 partitions (odd head landed
    via a tiny SBUF->SBUF DMA), so its matmuls contract 128 rows, and the
    four sc=0 units are spread one per group over groups 4-7.
  - PE warmup matmuls flip the HAM clock gate during the first DMAs; a
    dummy exp pulls the ACT table load (~2.7us) off the critical path;
    DMAs are ordered by first use on the sync HWDGE queue.

Host-side work is limited to layout (transpose/slice/concat), the exact
power-of-two weight prescale, and adding out_proj_bias (a zero vector per
the problem spec; in_proj biases are likewise zero and are not applied).
"""

import numpy as np

S, B, E = 2048, 4, 512
H, D = 8, 64
NCORES = 8
SC = S // 2          # query rows per core
T = S                # key rows (full)
NT = T // 128        # t-blocks of 128
KC = E // 128        # contraction chunks for projections
NG = 8               # attention groups: (sc-chunk, head-pair)
AV_LAG = 4           # AV emission lag (iters) behind QK for DVE-mul tiles
GPS_LAG = 5          # lag for gpsimd-mul tiles (hides Q7 latency)
GPS_T = (2, 5, 8, 11, 14)   # iterations whose A-multiply goes to gpsimd

_compiled = None


def _build():
    import concourse.bacc as bacc
    import concourse.mybir as mybir
    import concourse.tile as tile

    f32 = mybir.dt.float32
    bf16 = mybir.dt.bfloat16
    Exp = mybir.ActivationFunctionType.Exp
    Mult = mybir.AluOpType.mult

    nc = bacc.Bacc("TRN2", target_bir_lowering=False, debug=False)

    xq_d = nc.dram_tensor("xq_t", [E, SC], bf16, kind="ExternalInput")
    xk_d = nc.dram_tensor("xk_t", [E, T], bf16, kind="ExternalInput")
    xv_d = nc.dram_tensor("xv_t", [E, T], bf16, kind="ExternalInput")
    mask_d = nc.dram_tensor("mask_t", [T, SC], bf16, kind="ExternalInput")
    wq_d = nc.dram_tensor("wq_t", [E, E], bf16, kind="ExternalInput")
    wk_d = nc.dram_tensor("wk_t", [E, E], bf16, kind="ExternalInput")
    wv_d = nc.dram_tensor("wv_t", [E, E], bf16, kind="ExternalInput")
    wo_d = nc.dram_tensor("wo_t", [E, E], bf16, kind="ExternalInput")
    out_d = nc.dram_tensor("out", [SC, E], f32, kind="ExternalOutput")

    with tile.TileContext(nc) as tc:
        with (
            tc.tile_pool(name="persist", bufs=1) as pp,
            tc.tile_pool(name="mstage", bufs=3) as mst,
            tc.tile_pool(name="et", bufs=7) as etp,
            tc.tile_pool(name="a2", bufs=7) as a2p,
            tc.tile_pool(name="ao", bufs=8) as aop,
            tc.tile_pool(name="norm", bufs=2) as nrm,
            tc.tile_pool(name="osb", bufs=3) as osp,
            tc.tile_pool(name="sp", bufs=2, space="PSUM") as spp,
            tc.tile_pool(name="av", bufs=2, space="PSUM") as avp,
            tc.tile_pool(name="pj", bufs=2, space="PSUM") as pjp,
        ):
            # ---- persistent SBUF tensors ----
            wq = pp.tile([128, KC, E], bf16, tag="wq")
            wk = pp.tile([128, KC, E], bf16, tag="wk")
            wv = pp.tile([128, KC, E], bf16, tag="wv")
            # out-proj weights, head-PAIR major: pair q rows = heads 2q,2q+1
            wo2 = pp.tile([128, 4, E], bf16, tag="wo2")
            xq = pp.tile([128, KC, SC], bf16, tag="xq")
            xk = pp.tile([128, KC, T], bf16, tag="xk")
            xv = pp.tile([128, KC, T], bf16, tag="xv")
            g = pp.tile([128, NT, SC], bf16, tag="g")
            qT = pp.tile([128, KC, SC], bf16, tag="qT")
            kT = pp.tile([128, KC, T], bf16, tag="kT")
            vA = pp.tile([128, NT, H * 65], bf16, tag="vA")

            # ---- engine warmup: flip HAM to K=8/8 and pull the ACT ----
            # ---- exp-table load off the critical path, during lead-in DMAs
            ws = pp.tile([128, 512], bf16, tag="ws")
            nc.vector.memset(ws[:], 0)
            wact = pp.tile([1, 16], f32, tag="wact")
            nc.scalar.activation(wact[:], ws[0:1, 0:16], Exp)
            ones64 = pp.tile([1, 64], bf16, tag="ones64")
            nc.vector.memset(ones64[:], 1)

            def dma_x_act(dst, src, c2, width=512):
                # lead-in transfers ride the (otherwise idle) ACT HWDGE
                # queue, in parallel with the sync-queue stream
                sl = slice(c2 * width, (c2 + 1) * width)
                nc.scalar.dma_start(
                    out=dst[:, :, sl],
                    in_=src.ap().rearrange("(c p) s -> p c s", p=128)[:, :, sl],
                )

            def warm_mms(n):
                wps = pjp.tile([128, 512], f32, tag="pj")
                for _ in range(n):
                    nc.tensor.matmul(
                        wps[:], lhsT=ws[:, 0:128], rhs=ws[:], start=True, stop=True
                    )

            # trailing ones column for each head's fused rowsum
            for h in range(H):
                nc.vector.memset(vA[:, :, h * 65 + 64 : h * 65 + 65], 1)

            # ---- DMA schedule (one queue, ordered by first use) ----
            def dma_w(dst, src):
                nc.sync.dma_start(
                    out=dst[:], in_=src.ap().rearrange("(c p) e -> p c e", p=128)
                )

            def dma_x(dst, src, c2, width=512):
                sl = slice(c2 * width, (c2 + 1) * width)
                nc.sync.dma_start(
                    out=dst[:, :, sl],
                    in_=src.ap().rearrange("(c p) s -> p c s", p=128)[:, :, sl],
                )

            mtiles = []

            def dma_m(k):  # mask chunk of 2 t-blocks
                mt = mst.tile([128, 2, SC], bf16, tag="m")
                nc.sync.dma_start(
                    out=mt[:],
                    in_=mask_d.ap().rearrange("(c p) s -> p c s", p=128)[
                        :, 2 * k : 2 * k + 2, :
                    ],
                )
                mtiles.append(mt)

            # ordered by first use inside group 0 (whose pace is set by the
            # ACT stream at ~2.1us/iter); xq1 and wo are the only tensors
            # not needed until groups 2/4 and go last
            dma_w(wq, wq_d)
            dma_x(xq, xq_d, 0)
            dma_m(0)
            dma_m(1)
            dma_w(wk, wk_d)
            dma_x(xk, xk_d, 0)
            dma_w(wv, wv_d)
            dma_x(xv, xv_d, 0)
            dma_x(xk, xk_d, 1)
            dma_m(2)
            dma_x(xv, xv_d, 1)
            dma_m(3)
            dma_x(xk, xk_d, 2)
            dma_m(4)
            dma_x(xv, xv_d, 2)
            dma_x(xk, xk_d, 3)
            dma_m(5)
            dma_x(xv, xv_d, 3)
            dma_m(6)
            dma_m(7)
            dma_x(xq, xq_d, 1)
            nc.sync.dma_start(
                out=wo2[:], in_=wo_d.ap().rearrange("(q p) e -> p q e", p=128)
            )

            # ---- projection work units (drained during attention) ----
            def qT_unit(eo, c2):
                ps = pjp.tile([128, 512], f32, tag="pj")
                for kc in range(KC):
                    nc.tensor.matmul(
                        ps[:],
                        lhsT=wq[:, kc, eo * 128 : (eo + 1) * 128],
                        rhs=xq[:, kc, c2 * 512 : (c2 + 1) * 512],
                        start=(kc == 0),
                        stop=(kc == KC - 1),
                    )
                nc.vector.tensor_copy(
                    out=qT[:, eo, c2 * 512 : (c2 + 1) * 512], in_=ps[:]
                )

            def kT_unit(eo, c2):
                ps = pjp.tile([128, 512], f32, tag="pj")
                for kc in range(KC):
                    nc.tensor.matmul(
                        ps[:],
                        lhsT=wk[:, kc, eo * 128 : (eo + 1) * 128],
                        rhs=xk[:, kc, c2 * 512 : (c2 + 1) * 512],
                        start=(kc == 0),
                        stop=(kc == KC - 1),
                    )
                nc.vector.tensor_copy(
                    out=kT[:, eo, c2 * 512 : (c2 + 1) * 512], in_=ps[:]
                )

            def vN_unit(tb):
                ps = pjp.tile([128, 512], f32, tag="pj")
                for kc in range(KC):
                    nc.tensor.matmul(
                        ps[:],
                        lhsT=xv[:, kc, tb * 128 : (tb + 1) * 128],
                        rhs=wv[:, kc, :],
                        start=(kc == 0),
                        stop=(kc == KC - 1),
                    )
                nc.vector.tensor_copy(
                    out=vA[:, tb, :].rearrange("p (h c) -> p h c", h=H)[:, :, 0:64],
                    in_=ps[:].rearrange("p (h c) -> p h c", h=H),
                )

            ao_tiles = {}  # (sc, q) -> [128, 512] bf16 normalized attn out
            #               head pair 2q stacked on partitions (even 0:64,
            #               odd 64:128) so out-proj contracts 128 at a time

            def op_unit(sc, blk):
                ps = pjp.tile([128, 512], f32, tag="pj")
                for hp in range(4):
                    nc.tensor.matmul(
                        ps[:],
                        lhsT=ao_tiles[(sc, hp)][:, blk * 128 : (blk + 1) * 128],
                        rhs=wo2[:, hp, :],
                        start=(hp == 0),
                        stop=(hp == 3),
                    )
                osb = osp.tile([128, 512], f32, tag="osb")
                nc.vector.tensor_copy(out=osb[:], in_=ps[:])
                r0 = sc * 512 + blk * 128
                nc.sync.dma_start(out=out_d.ap()[r0 : r0 + 128, :], in_=osb[:])

            # per-(group, iter) projection drain plan.  vN(tb) is placed ~2
            # iters after its xv chunk lands (DMA-paced in group 0); kT(0,c)
            # likewise trails the xk chunk c arrival and leads QK t=4c.
            plan = [dict() for _ in range(NG)]
            plan[0] = {
                3: [("vN", 0), ("kT", 0, 1)], 4: [("vN", 1)],
                5: [("vN", 2)], 6: [("vN", 3), ("kT", 0, 2)],
                7: [("vN", 4)], 8: [("vN", 5), ("kT", 0, 3)],
                9: [("vN", 6)], 10: [("vN", 7)],
                11: [("vN", 8), ("kT", 1, 0)], 12: [("vN", 9), ("qT", 1, 0)],
                13: [("vN", 10), ("kT", 1, 1)],
                14: [("vN", 11), ("kT", 1, 2)], 15: [("vN", 12), ("kT", 1, 3)],
            }
            plan[1] = {
                0: [("vN", 13)], 1: [("vN", 14)], 2: [("vN", 15)],
                4: [("kT", 2, 0)], 6: [("qT", 2, 0)], 8: [("kT", 2, 1)],
                10: [("kT", 2, 2)], 12: [("kT", 2, 3)],
            }
            plan[2] = {
                0: [("kT", 3, 0)], 2: [("kT", 3, 1)], 4: [("kT", 3, 2)],
                6: [("kT", 3, 3)], 8: [("qT", 3, 0)], 10: [("qT", 0, 1)],
                12: [("qT", 1, 1)],
            }
            plan[3] = {2: [("qT", 2, 1)], 4: [("qT", 3, 1)]}
            plan[4] = {6: [("op", 0, 0)]}
            plan[5] = {6: [("op", 0, 1)]}
            plan[6] = {6: [("op", 0, 2)]}
            plan[7] = {6: [("op", 0, 3)]}

            def run_unit(u):
                if u[0] == "vN":
                    vN_unit(u[1])
                elif u[0] == "kT":
                    kT_unit(u[1], u[2])
                elif u[0] == "qT":
                    qT_unit(u[1], u[2])
                elif u[0] == "op":
                    op_unit(u[1], u[2])

            # ---- decoupled group epilogue (normalize by fused rowsum) ----
            # Emitted as 6 steps spread over the first iterations of the
            # NEXT group so no engine queue head-of-line blocks on it.
            # gpsimd must NOT appear here — partition_broadcast lives in a
            # different Q7 library than tensor_tensor, and each library
            # switch costs ~6-8us of invisible IRAM reload.  The partition
            # broadcast is a K=1 PE matmul (ones-row ⊗ recip-row) into a
            # 64-row PSUM tile instead.
            def epi_steps(av, sc, h0):
                state = {}

                def s_copy(j):
                    def run():
                        rr = nrm.tile([96, 512], f32, tag="rr")
                        nc.vector.tensor_copy(out=rr[64:65, :], in_=av[j][64:65, :])
                        # rowsum row 64 -> rows 0:32 (cross-quadrant shuffle);
                        # DVE ops stay lane-aligned, so recip maps row 0 ->
                        # row 0 of a fresh tile
                        nc.vector.stream_shuffle(rr[0:32, :], rr[64:96, :], [0] * 32)
                        # AV numerator to SBUF (frees the PSUM accumulator;
                        # also the normalize multiply may read only one PSUM
                        # operand, and the recip broadcast below is in PSUM)
                        avs = nrm.tile([64, 512], bf16, tag="avs")
                        nc.vector.tensor_copy(out=avs[:], in_=av[j][0:64, :])
                        state[("rr", j)] = rr
                        state[("avs", j)] = avs

                    return run

                def s_recip(j):
                    def run():
                        r1 = nrm.tile([1, 512], f32, tag="r1")
                        nc.vector.reciprocal_approx_fast(
                            out=r1[:], in_=state[("rr", j)][0:1, :]
                        )
                        r1b = nrm.tile([1, 512], bf16, tag="r1b")
                        nc.vector.tensor_copy(out=r1b[:], in_=r1[:])
                        # broadcast the recip row to 64 partitions with a
                        # K=1 PE matmul (ones-column ⊗ recip-row -> PSUM)
                        rb = pjp.tile([128, 512], f32, tag="pj")
                        nc.tensor.matmul(
                            rb[0:64, :],
                            lhsT=ones64[:],
                            rhs=r1b[:],
                            start=True,
                            stop=True,
                        )
                        state[("rb", j)] = rb

                    return run

                def s_ao(j):
                    def run():
                        if j == 0:
                            aoP = aop.tile([128, 512], bf16, tag="aoP")
                            state["aoP"] = aoP
                            ao_tiles[(sc, h0 // 2)] = aoP
                            nc.vector.tensor_tensor(
                                out=aoP[0:64, :],
                                in0=state[("avs", 0)][:],
                                in1=state[("rb", 0)][0:64, :],
                                op=Mult,
                            )
                        else:
                            # odd head: normalize into a staging tile, then a
                            # tiny SBUF->SBUF DMA lands it on partitions
                            # 64:128 of the pair tile (DVE stays lane-aligned)
                            aot = aop.tile([64, 512], bf16, tag="aoT")
                            nc.vector.tensor_tensor(
                                out=aot[:],
                                in0=state[("avs", 1)][:],
                                in1=state[("rb", 1)][0:64, :],
                                op=Mult,
                            )
                            nc.sync.dma_start(
                                out=state["aoP"][64:128, :], in_=aot[:]
                            )

                    return run

                return [
                    s_copy(0), s_recip(0), s_copy(1),
                    s_ao(0), s_recip(1), s_ao(1),
                ]

            # ---- lead-in: warm MMs bridge to the first projections ----
            warm_mms(10)
            qT_unit(0, 0)
            kT_unit(0, 0)

            # ---- attention: 8 groups of (sc-chunk, head-pair) ----
            # Work that trails its group (lagged AV matmuls, the normalize
            # epilogue) is queued with a global due-iteration and flushed at
            # the top of later iterations, so no engine queue ever
            # head-of-line blocks at a group boundary.
            prep_done = 0
            pend_av = []   # (due abs-iter, seq, tb, a2 tile, group ctx)
            pend_epi = []  # (due abs-iter, seq, step fn)
            seq = 0

            class Gctx:
                def __init__(self, gi):
                    self.h0 = 2 * (gi % 4)
                    self.av = [None, None]
                    self.n_av = 0

                def emit_av(self, tb, a2):
                    if self.av[0] is None:
                        for j in range(2):
                            self.av[j] = avp.tile(
                                [128, 512], f32, tag="av", name=f"av{j}"
                            )
                    for j in range(2):
                        nc.tensor.matmul(
                            self.av[j][0:65, :],
                            lhsT=vA[
                                :, tb, (self.h0 + j) * 65 : (self.h0 + j + 1) * 65
                            ],
                            rhs=a2[:, j * 512 : (j + 1) * 512],
                            start=(self.n_av == 0),
                            stop=(self.n_av == NT - 1),
                            skip_group_check=True,
                        )
                    self.n_av += 1

            def flush(now):
                for entry in sorted(pend_av):
                    if entry[0] <= now:
                        pend_av.remove(entry)
                        entry[4].emit_av(entry[2], entry[3])
                for entry in sorted(pend_epi):
                    if entry[0] <= now:
                        pend_epi.remove(entry)
                        entry[2]()

            for gi in range(NG):
                sc, q = gi // 4, gi % 4
                ssl = slice(sc * 512, (sc + 1) * 512)
                ctx = Gctx(gi)

                for t in range(NT):
                    A = gi * NT + t
                    # QK pair (row-quadrant packed, concurrent)
                    sp = spp.tile([128, 1024], f32, tag="sp")
                    for j in range(2):
                        jsl = slice(j * 64, (j + 1) * 64)
                        nc.tensor.matmul(
                            sp[:, j * 512 : (j + 1) * 512],
                            lhsT=kT[jsl, q, t * 128 : (t + 1) * 128],
                            rhs=qT[jsl, q, ssl],
                            start=True,
                            stop=True,
                            tile_position=(j * 64, 0),
                        )
                    # exp(mask) k=0 goes before the very first score exp
                    # (its DMA lands first); later chunks go after the score
                    # exp so they never head-of-line block it
                    if gi == 0 and t == 0:
                        nc.scalar.activation(g[:, 0:2, :], mtiles[0][:], Exp)
                        prep_done += 1
                    # exp(scores)
                    et = etp.tile([128, 1024], bf16, tag="et")
                    nc.scalar.activation(et[:], sp[:], Exp)
                    if gi == 0 and t % 2 == 1 and prep_done < NT // 2:
                        k = (t + 1) // 2
                        nc.scalar.activation(
                            g[:, 2 * k : 2 * k + 2, :], mtiles[k][:], Exp
                        )
                        prep_done += 1
                    # A = exp(scores) * exp(mask), mask factor doubled across
                    # the two heads via a stride-0 broadcast
                    a2 = a2p.tile([128, 1024], bf16, tag="a2")
                    gb = g[:, t, ssl].unsqueeze(1).broadcast_to([128, 2, 512])
                    on_gps = t in (GPS_T if gi < NG - 1 else (2, 5, 8, 11))
                    eng = nc.gpsimd if on_gps else nc.vector
                    eng.tensor_tensor(
                        out=a2[:].rearrange("p (a b) -> p a b", a=2),
                        in0=et[:].rearrange("p (a b) -> p a b", a=2),
                        in1=gb,
                        op=Mult,
                    )
                    pend_av.append(
                        (A + (GPS_LAG if on_gps else AV_LAG), seq, t, a2, ctx)
                    )
                    seq += 1
                    # lagged AV / carried epilogue work
                    flush(A)
                    # drain projection work
                    for u in plan[gi].get(t, ()):
                        run_unit(u)

                # queue this group's normalize epilogue: all its AVs land by
                # abs due (gi*NT + 15 + AV_LAG); epilogue right after
                due = gi * NT + NT - 1 + AV_LAG
                for st in epi_steps(ctx.av, sc, ctx.h0):
                    pend_epi.append((due, seq, st))
                    seq += 1

            # ---- tail: drain carried work, then out-proj for sc=1 ----
            flush(10**9)
            for blk in range(4):
                op_unit(1, blk)

    nc.compile()
    return nc


def _get_compiled():
    global _compiled
    if _compiled is None:
        _compiled = _build()
    return _compiled


def _prep_in_maps(query, key, value, attn_mask, in_proj_weight):
    import ml_dtypes

    bf = ml_dtypes.bfloat16
    q_t = np.ascontiguousarray(query.transpose(1, 2, 0).astype(bf))   # [B, E, S]
    k_t = np.ascontiguousarray(key.transpose(1, 2, 0).astype(bf))
    v_t = np.ascontiguousarray(value.transpose(1, 2, 0).astype(bf))
    m_t = np.ascontiguousarray(attn_mask.transpose(0, 2, 1).astype(bf))  # [B,T,S]
    # 1/sqrt(D) = 1/8 folded into Wq -- exact in fp32 (power of two)
    wq_t = np.ascontiguousarray((in_proj_weight[0:E] * 0.125).T.astype(bf))
    wk_t = np.ascontiguousarray(in_proj_weight[E : 2 * E].T.astype(bf))
    wv_t = np.ascontiguousarray(in_proj_weight[2 * E : 3 * E].T.astype(bf))
    in_maps = []
    for c in range(NCORES):
        b, hf = c // 2, c % 2
        sl = slice(hf * SC, (hf + 1) * SC)
        in_maps.append(
            {
                "xq_t": np.ascontiguousarray(q_t[b][:, sl]),
                "xk_t": k_t[b],
                "xv_t": v_t[b],
                "mask_t": np.ascontiguousarray(m_t[b][:, sl]),
                "wq_t": wq_t,
                "wk_t": wk_t,
                "wv_t": wv_t,
            }
        )
    return in_maps


def kernel(
    query,
    key,
    value,
    attn_mask,
    in_proj_weight,
    in_proj_bias,
    out_proj_weight,
    out_proj_bias,
):
    from concourse.bass_utils import run_bass_kernel_spmd

    query = np.asarray(query, np.float32)
    key = np.asarray(key, np.float32)
    value = np.asarray(value, np.float32)
    attn_mask = np.asarray(attn_mask, np.float32)
    in_proj_weight = np.asarray(in_proj_weight, np.float32)
    out_proj_weight = np.asarray(out_proj_weight, np.float32)
    out_proj_bias = np.asarray(out_proj_bias, np.float32)

    nc = _get_compiled()
    in_maps = _prep_in_maps(query, key, value, attn_mask, in_proj_weight)
    import ml_dtypes

    wo_t = np.ascontiguousarray(out_proj_weight.T.astype(ml_dtypes.bfloat16))
    for m in in_maps:
        m["wo_t"] = wo_t

    res = run_bass_kernel_spmd(nc, in_maps, core_ids=list(range(NCORES)))

    out = np.empty((S, B, E), np.float32)
    for c in range(NCORES):
        b, hf = c // 2, c % 2
        out[hf * SC : (hf + 1) * SC, b, :] = res.results[c]["out"]
    # out_proj_bias is zeros per the problem spec; adding it on the host is
    # exact. (in_proj biases are also zeros and are not applied on-device.)
    out += out_proj_bias[None, None, :]
    return out
